# revision 9
# baseline (speedup 1.0000x reference)
"""Trainium2 Bass kernel for the Centroid (segment_reduce) problem.

new_centroid = 0.3 * (segment_sum(embed, y) / counts) + 0.7 * centroid
  embed [32768, 1024] f32, y [32768] int (0..999), centroid [1000, 1024] f32

Strategy (8 NeuronCores, CLASS-parallel via host-side routing):
  - The host partitions the 1000 classes into 8 groups of <=128 classes,
    balanced by sample count (LPT + swap refinement; for the uniform
    label distribution every group lands at ~4096 of the 32768 samples).
  - Core i receives ONLY the embed rows whose label falls in its group
    (as fp8 e4m3 with a trailing constant 1.0 column, padded with zero
    rows to a fixed CAP). Each core fully owns its classes so there is
    NO collective at all.
  - On device the scatter-add is a one-hot matmul on TensorE in fp8
    DoubleRow mode with a SINGLE 128-slot class tile:
        sums[slot, d] = sum_b onehot[b, slot] * embed[b, d]
    The ones column makes the per-slot count fall out of the same
    matmuls (pad rows have an all-zero one-hot row, so they contribute
    neither sums nor counts).
  - epilogue per core: out = sums * (0.3/count) + 0.7*centroid for the
    core's <=128 slots; the host scatters slot rows back to class rows.

Schedule notes (v4): the slot labels for ALL k-tiles ride inside the
FIRST embed tile (as bf16 pairs in its alignment pad, bitcast on
device), so no separate label DMA exists -- the baseline's tiny
128B-per-partition label transfer cost ~6us of descriptor-bound DMA
and gated every one-hot build.  Embed pair-tiles are round-robined
across the sync/scalar/gpsimd queues in consumption order so landing
order matches matmul order and the three hardware queues saturate the
~358 GB/s HBM ceiling; 0.7*centroid is pre-multiplied on the host
(bf16) and lands late on scalar, off the critical path.  The epilogue
splits the per-slot scale multiplies across ACT and DVE in parallel
and ships the output with two DMAs on two queues.
"""

import numpy as np

import concourse.bacc as bacc
import concourse.mybir as mybir
import concourse.tile as tile
from concourse.bass_utils import run_bass_kernel_spmd

N_CORES = 8
C = 1000  # real classes
D = 1024  # embed dim
W = 1040  # embed + ones column at 1024 + zero pad (16B-aligned rows)
W0 = 1104  # first-tile k-tile stride: W + 64B of f32 slot labels
B = 32768  # total batch
P = 128
FACTOR = 0.3
# matmul column chunks (PSUM bank limit is 512 f32); the counts chunk
# (dims 896..1023 + the ones column at 1024 + pad) is computed FIRST per
# pair so the reciprocal can start as early as possible at the end
CHUNKS = [(896, 144), (0, 448), (448, 448)]

_F32 = mybir.dt.float32
_BF16 = mybir.dt.bfloat16
_FP8 = mybir.dt.float8e4

_CACHE: dict = {}


def _build(cap: int):
    kt = cap // P  # k-tiles per core
    kp = kt // 2  # k-pairs; DoubleRow consumes [128, 2, cols] per matmul

    nc = bacc.Bacc(
        "TRN2", target_bir_lowering=False, debug=False, num_devices=N_CORES
    )
    # emb0[p, j2, :] = k-tiles 0,1; cols 1040:1104 of sub-block j2 hold
    # f32 slot labels for k-tiles j2*16 .. j2*16+15 (label of padded row
    # k*128+p, -1.0 for pads)
    emb0 = nc.dram_tensor("emb0", [P, 2, W0], _FP8, kind="ExternalInput").ap()
    # embr[p, k, :] = padded_rows[(k+2)*128 + p, :]; col D is constant 1.0
    embr = nc.dram_tensor("embr", [P, kt - 2, W], _FP8, kind="ExternalInput").ap()
    # cent07 = 0.7 * centroid rows for this core's slots (bf16)
    cent = nc.dram_tensor("cent", [P, D], _BF16, kind="ExternalInput").ap()
    out = nc.dram_tensor("out", [P, D], _BF16, kind="ExternalOutput").ap()

    with tile.TileContext(nc) as tc:
        with (
            tc.tile_pool(name="const", bufs=1) as const_pool,
            tc.tile_pool(name="emb0p", bufs=1) as emb0_pool,
            tc.tile_pool(name="emb", bufs=kp - 1) as emb_pool,
            tc.tile_pool(name="oh", bufs=kp) as oh_pool,
            tc.tile_pool(name="psum", bufs=1, space="PSUM") as psum_pool,
            tc.tile_pool(name="fin", bufs=1) as fin_pool,
        ):
            # first embed tile (with embedded labels) goes out before
            # anything else -- it gates the one-hot builds AND the first
            # matmul
            emb_t0 = emb0_pool.tile([P, 2, W0], _FP8, name="emb0")
            nc.sync.dma_start(out=emb_t0[:], in_=emb0[:])

            # embed pair-tiles round-robin across the three DMA queues in
            # consumption order (pair j on queue j%3; pair 0 is emb_t0 on
            # sync) so landing order matches matmul order; queue byte
            # totals balanced: pair 15 rides gpsimd, cent mid-scalar
            emb_qs = {0: nc.sync, 1: nc.scalar, 2: nc.gpsimd}
            emb_tiles = [emb_t0]
            cent_sb = fin_pool.tile([P, D], _BF16, name="cent_sb")
            iota = const_pool.tile([P, P], _F32)
            for j in range(1, kp):
                emb_t = emb_pool.tile([P, 2, W], _FP8, name=f"emb{j}", tag="emb")
                q = 2 if j == kp - 1 else j % 3
                emb_qs[q].dma_start(
                    out=emb_t[:], in_=embr[:, 2 * j - 2 : 2 * j, :]
                )
                emb_tiles.append(emb_t)
                if j == 2:
                    # iota row replicated down all 128 partitions:
                    # iota[p, s] = s (after gpsimd's first DMA issue so it
                    # doesn't delay the gpsimd queue start)
                    nc.gpsimd.iota(
                        iota[:],
                        pattern=[[1, P]],
                        base=0,
                        channel_multiplier=0,
                        allow_small_or_imprecise_dtypes=True,
                    )
                if j == 7:
                    # 0.7*centroid (host-premultiplied, bf16) mid-scalar:
                    # lands well before the epilogue adds need it, without
                    # displacing the late embed pairs
                    nc.scalar.dma_start(out=cent_sb[:], in_=cent[:])

            psums = [
                psum_pool.tile([P, n], _F32, name=f"ps{q}")
                for q, (_, n) in enumerate(CHUNKS)
            ]

            # one-hot builds: all depend only on emb_t0 (labels) + iota,
            # so DVE streams through them well ahead of the matmuls
            oh_tiles = []
            for j in range(kp):
                oh_t = oh_pool.tile([P, 2, P], _FP8, name=f"oh{j}", tag="oh")
                for j2 in range(2):
                    k = 2 * j + j2
                    ysl = emb_t0[
                        :, k // 16, W + 4 * (k % 16) : W + 4 * (k % 16) + 4
                    ].bitcast(_F32)
                    nc.vector.tensor_scalar(
                        oh_t[:, j2, :],
                        iota[:],
                        ysl,
                        None,
                        mybir.AluOpType.is_equal,
                    )
                oh_tiles.append(oh_t)

            for j in range(kp):
                emb_t = emb_tiles[j]
                for q, (lo, n) in enumerate(CHUNKS):
                    nc.tensor.matmul(
                        psums[q][:],
                        lhsT=oh_tiles[j][:],
                        rhs=emb_t[:, :, lo : lo + n],
                        start=(j == 0),
                        stop=(j == kp - 1),
                        perf_mode=mybir.MatmulPerfMode.DoubleRow,
                    )

            # epilogue: r3 = 0.3/count, out = sums*r3 + 0.7*centroid.
            # scale-multiplies split across ACT (chunks 1, 0) and DVE
            # (chunk 2) in parallel, bf16 outputs so the DVE adds run in
            # 16-bit mode (2x); each output chunk ships as soon as its add
            # lands, split across the sync and scalar queues.
            r3 = fin_pool.tile([P, 1], _F32, name="r3")
            nc.vector.reciprocal(r3[:], psums[0][:, 128:129])
            nc.vector.tensor_scalar(
                r3[:], r3[:], FACTOR, None, mybir.AluOpType.mult
            )
            out_sb = fin_pool.tile([P, D], _BF16, name="out_sb")
            t1 = fin_pool.tile([P, 448], _BF16, name="t1")
            t2 = fin_pool.tile([P, 448], _BF16, name="t2")
            t0c = fin_pool.tile([P, 128], _BF16, name="t0c")
            # chunk 1 (cols 0:448) scale on ACT
            nc.scalar.mul(t1[:], psums[1][:, 0:448], r3[:, 0:1])
            # chunk 2 (cols 448:896) scale on DVE
            nc.vector.tensor_scalar(
                t2[:], psums[2][:, 0:448], r3[:, 0:1], None, mybir.AluOpType.mult
            )
            # chunk 0 (cols 896:1024) scale on ACT
            nc.scalar.mul(t0c[:], psums[0][:, 0:128], r3[:, 0:1])
            nc.vector.tensor_tensor(
                out=out_sb[:, 0:448],
                in0=t1[:],
                in1=cent_sb[:, 0:448],
                op=mybir.AluOpType.add,
            )
            nc.sync.dma_start(out=out[:, 0:448], in_=out_sb[:, 0:448])
            nc.vector.tensor_tensor(
                out=out_sb[:, 448:896],
                in0=t2[:],
                in1=cent_sb[:, 448:896],
                op=mybir.AluOpType.add,
            )
            nc.scalar.dma_start(out=out[:, 448:896], in_=out_sb[:, 448:896])
            nc.vector.tensor_tensor(
                out=out_sb[:, 896:D],
                in0=t0c[:],
                in1=cent_sb[:, 896:D],
                op=mybir.AluOpType.add,
            )
            nc.sync.dma_start(out=out[:, 896:D], in_=out_sb[:, 896:D])

    nc.compile()
    return nc


def get_nc(cap: int = 4096):
    if cap not in _CACHE:
        _CACHE[cap] = _build(cap)
    return _CACHE[cap]


def _refine(groups, sums, counts, target):
    """2-opt repair: swap classes between the max bin and any other bin
    whenever it strictly lowers max(pair); stop at max <= target."""
    for _ in range(6000):
        hi = int(np.argmax(sums))
        if sums[hi] <= target:
            return True
        best = None  # (new_pair_max, ci, cj, b, d)
        for b in range(N_CORES):
            if b == hi:
                continue
            for ci in groups[hi]:
                for cj in groups[b]:
                    d = int(counts[ci]) - int(counts[cj])
                    if d <= 0:
                        continue
                    m = max(sums[hi] - d, sums[b] + d)
                    if m < sums[hi] and (best is None or m < best[0]):
                        best = (m, ci, cj, b, d)
        if best is None:
            return False
        _m, ci, cj, b, d = best
        groups[hi].remove(ci)
        groups[b].remove(cj)
        groups[hi].append(cj)
        groups[b].append(ci)
        sums[hi] -= d
        sums[b] += d
    return bool(np.max(sums) <= target)


def _partition_classes(counts: np.ndarray):
    """Split classes into N_CORES groups, <=128 classes each, minimizing
    the max total sample count. LPT greedy + 2-opt repair, with a few
    deterministic randomized restarts to reach a perfect equipartition."""
    target = int(np.ceil(counts.sum() / N_CORES))
    order = np.argsort(-counts, kind="stable")
    best_groups, best_sums = None, None
    for seed in range(8):
        rng = np.random.default_rng(seed)
        groups = [[] for _ in range(N_CORES)]
        sums = np.zeros(N_CORES, dtype=np.int64)
        for c in order:
            cand = np.argsort(
                sums + (rng.integers(0, 2, N_CORES) if seed else 0),
                kind="stable",
            )
            for b in cand:
                if len(groups[b]) < P:
                    groups[b].append(int(c))
                    sums[b] += counts[c]
                    break
        ok = _refine(groups, sums, counts, target)
        if best_sums is None or sums.max() < best_sums.max():
            best_groups, best_sums = groups, sums
        if ok:
            break
    return best_groups, best_sums


def make_in_maps(embed: np.ndarray, y: np.ndarray, centroid: np.ndarray):
    fp8_np = mybir.dt.np(_FP8)
    bf16_np = mybir.dt.np(_BF16)
    embed8 = np.ascontiguousarray(embed, dtype=np.float32).astype(fp8_np)
    y = np.asarray(y).astype(np.int64)
    centroid = np.asarray(centroid, dtype=np.float32)
    counts = np.bincount(y, minlength=C)

    groups, sums = _partition_classes(counts)
    cap = max(4096, int(np.ceil(sums.max() / 256.0)) * 256)

    # class -> (core, slot) map
    core_of = np.full(C, -1, dtype=np.int64)
    slot_of = np.full(C, -1, dtype=np.int64)
    for i, g in enumerate(groups):
        for s, cls in enumerate(g):
            core_of[cls] = i
            slot_of[cls] = s

    kt = cap // P
    in_maps = []
    meta = []
    for i in range(N_CORES):
        rows = np.nonzero(core_of[y] == i)[0]
        n = rows.shape[0]
        emb_pad = np.zeros((cap, W), dtype=fp8_np)
        emb_pad[:n, :D] = embed8[rows]
        emb_pad[:, D] = 1.0  # counts column (pad rows are masked by onehot)
        # cols D+1..W-1 stay zero (row alignment pad)
        ys = np.full(cap, -1.0, dtype=np.float32)
        ys[:n] = slot_of[y[rows]].astype(np.float32)
        # ysb[p, k] = slot label of padded row k*128+p, in f32
        ysb = np.ascontiguousarray(ys.reshape(kt, P).T)  # [P, kt] f32
        # emb8[p, k, :] = emb_pad[k*128 + p, :]
        emb8 = emb_pad.reshape(kt, P, W).transpose(1, 0, 2)
        # first tile: k-tiles 0,1 + all kt f32 labels in the pad tail.
        # sub-block j2 carries labels for k-tiles j2*16 .. j2*16+15.
        emb0 = np.zeros((P, 2, W0), dtype=fp8_np)
        emb0[:, :, :W] = emb8[:, 0:2, :]
        emb0[:, 0, W:W0] = ysb[:, 0:16].view(np.uint8).view(fp8_np)
        emb0[:, 1, W:W0] = ysb[:, 16:32].view(np.uint8).view(fp8_np)
        cent_i = np.zeros((P, D), dtype=np.float32)
        g = groups[i]
        cent_i[: len(g)] = (1.0 - FACTOR) * centroid[g]
        in_maps.append(
            {
                "emb0": emb0,
                "embr": np.ascontiguousarray(emb8[:, 2:, :]),
                "cent": cent_i.astype(bf16_np),
            }
        )
        meta.append(g)
    return in_maps, meta, cap


def kernel(embed: np.ndarray, y: np.ndarray, centroid: np.ndarray) -> np.ndarray:
    in_maps, meta, cap = make_in_maps(embed, y, centroid)
    nc = get_nc(cap)
    res = run_bass_kernel_spmd(nc, in_maps, core_ids=list(range(N_CORES)))
    full = np.zeros((C, D), dtype=np.float32)
    for i in range(N_CORES):
        g = meta[i]
        full[g] = res.results[i]["out"][: len(g)].astype(np.float32)
    return full


# revision 11
# speedup vs baseline: 1.1235x; 1.1235x over previous
"""Trainium2 Bass kernel for the Centroid (segment_reduce) problem.

new_centroid = 0.3 * (segment_sum(embed, y) / counts) + 0.7 * centroid
  embed [32768, 1024] f32, y [32768] int (0..999), centroid [1000, 1024] f32

Strategy (8 NeuronCores, CLASS-parallel via host-side routing):
  - The host partitions the 1000 classes into 8 groups of <=128 classes,
    balanced by sample count (LPT + swap refinement; for the uniform
    label distribution every group lands at ~4096 of the 32768 samples).
  - Core i receives ONLY the embed rows whose label falls in its group
    (as fp8 e4m3 with a trailing constant 1.0 column, padded with zero
    rows to a fixed CAP). Each core fully owns its classes so there is
    NO collective at all.
  - On device the scatter-add is a one-hot matmul on TensorE in fp8
    DoubleRow mode with a SINGLE 128-slot class tile:
        sums[slot, d] = sum_b onehot[b, slot] * embed[b, d]
    The ones column makes the per-slot count fall out of the same
    matmuls (pad rows have an all-zero one-hot row, so they contribute
    neither sums nor counts).
  - epilogue per core: out = sums * (0.3/count) + 0.7*centroid for the
    core's <=128 slots; the host scatters slot rows back to class rows.

Schedule notes (v4): the slot labels for ALL k-tiles ride inside the
FIRST embed tile (as bf16 pairs in its alignment pad, bitcast on
device), so no separate label DMA exists -- the baseline's tiny
128B-per-partition label transfer cost ~6us of descriptor-bound DMA
and gated every one-hot build.  Embed pair-tiles are round-robined
across the sync/scalar/gpsimd queues in consumption order so landing
order matches matmul order and the three hardware queues saturate the
~358 GB/s HBM ceiling; 0.7*centroid is pre-multiplied on the host
(bf16) and lands late on scalar, off the critical path.  The epilogue
splits the per-slot scale multiplies across ACT and DVE in parallel
and ships the output with two DMAs on two queues.
"""

import numpy as np

import concourse.bacc as bacc
import concourse.mybir as mybir
import concourse.tile as tile
from concourse.bass_utils import run_bass_kernel_spmd

N_CORES = 8
C = 1000  # real classes
D = 1024  # embed dim
W = 1040  # embed + ones column at 1024 + zero pad (16B-aligned rows)
W0 = 1104  # first-tile k-tile stride: W + 64B of f32 slot labels
B = 32768  # total batch
P = 128
FACTOR = 0.3
# matmul column chunks (PSUM bank limit is 512 f32); the counts chunk
# (dims 896..1023 + the ones column at 1024 + pad) is computed FIRST per
# pair so the reciprocal can start as early as possible at the end
CHUNKS = [(896, 144), (0, 448), (448, 448)]

_F32 = mybir.dt.float32
_BF16 = mybir.dt.bfloat16
_FP8 = mybir.dt.float8e4

_CACHE: dict = {}


def _build(cap: int):
    kt = cap // P  # k-tiles per core
    kp = kt // 2  # k-pairs; DoubleRow consumes [128, 2, cols] per matmul

    nc = bacc.Bacc(
        "TRN2", target_bir_lowering=False, debug=False, num_devices=N_CORES
    )
    # emb0[p, j2, :] = k-tiles 0,1; cols 1040:1104 of sub-block j2 hold
    # f32 slot labels for k-tiles j2*16 .. j2*16+15 (label of padded row
    # k*128+p, -1.0 for pads)
    emb0 = nc.dram_tensor("emb0", [P, 2, W0], _FP8, kind="ExternalInput").ap()
    # embr[p, k, :] = padded_rows[(k+2)*128 + p, :]; col D is constant 1.0
    embr = nc.dram_tensor("embr", [P, kt - 2, W], _FP8, kind="ExternalInput").ap()
    # cent07 = 0.7 * centroid rows for this core's slots (bf16)
    cent = nc.dram_tensor("cent", [P, D], _BF16, kind="ExternalInput").ap()
    out = nc.dram_tensor("out", [P, D], _BF16, kind="ExternalOutput").ap()

    with tile.TileContext(nc) as tc:
        with (
            tc.tile_pool(name="const", bufs=1) as const_pool,
            tc.tile_pool(name="emb0p", bufs=1) as emb0_pool,
            tc.tile_pool(name="emb", bufs=kp - 1) as emb_pool,
            tc.tile_pool(name="oh", bufs=kp) as oh_pool,
            tc.tile_pool(name="psum", bufs=1, space="PSUM") as psum_pool,
            tc.tile_pool(name="fin", bufs=1) as fin_pool,
        ):
            # first embed tile (with embedded labels) goes out before
            # anything else -- it gates the one-hot builds AND the first
            # matmul
            emb_t0 = emb0_pool.tile([P, 2, W0], _FP8, name="emb0")
            nc.sync.dma_start(out=emb_t0[:], in_=emb0[:])

            # embed pair-tiles spread across the three DMA queues in
            # consumption order, with bytes proportional to each queue's
            # measured share of HBM bandwidth under 3-way contention
            # (gpsimd/SWDGE sustains ~1.75x the per-HWDGE-queue rate):
            # gpsimd takes every other pair (8), sync and scalar take the
            # rest; cent rides mid-scalar.
            #   gpsimd: p1 p3 p5 p7 p9 p11 p13 p15   (2.13 MB)
            #   sync:   emb0 p4 p8 p12 (+outs)       (1.08 MB)
            #   scalar: p2 p6 p10 cent p14           (1.32 MB)
            q_of = {
                1: nc.gpsimd, 3: nc.gpsimd, 5: nc.gpsimd, 7: nc.gpsimd,
                9: nc.gpsimd, 11: nc.gpsimd, 13: nc.gpsimd, 15: nc.gpsimd,
                4: nc.sync, 8: nc.sync, 12: nc.sync,
                2: nc.scalar, 6: nc.scalar, 10: nc.scalar, 14: nc.scalar,
            }
            emb_tiles = [emb_t0]
            cent_sb = fin_pool.tile([P, D], _BF16, name="cent_sb")
            iota = const_pool.tile([P, P], _F32)
            for j in range(1, kp):
                emb_t = emb_pool.tile([P, 2, W], _FP8, name=f"emb{j}", tag="emb")
                q_of[j].dma_start(
                    out=emb_t[:], in_=embr[:, 2 * j - 2 : 2 * j, :]
                )
                emb_tiles.append(emb_t)
                if j == 1:
                    # iota row replicated down all 128 partitions:
                    # iota[p, s] = s (after gpsimd's first DMA issue so it
                    # doesn't delay the gpsimd queue start)
                    nc.gpsimd.iota(
                        iota[:],
                        pattern=[[1, P]],
                        base=0,
                        channel_multiplier=0,
                        allow_small_or_imprecise_dtypes=True,
                    )
                if j == 10:
                    # 0.7*centroid (host-premultiplied, bf16) mid-scalar:
                    # lands well before the epilogue adds need it, without
                    # displacing the late embed pairs
                    nc.scalar.dma_start(out=cent_sb[:], in_=cent[:])

            psums = [
                psum_pool.tile([P, n], _F32, name=f"ps{q}")
                for q, (_, n) in enumerate(CHUNKS)
            ]

            # one-hot builds: all depend only on emb_t0 (labels) + iota,
            # so DVE streams through them well ahead of the matmuls
            oh_tiles = []
            for j in range(kp):
                oh_t = oh_pool.tile([P, 2, P], _FP8, name=f"oh{j}", tag="oh")
                for j2 in range(2):
                    k = 2 * j + j2
                    ysl = emb_t0[
                        :, k // 16, W + 4 * (k % 16) : W + 4 * (k % 16) + 4
                    ].bitcast(_F32)
                    nc.vector.tensor_scalar(
                        oh_t[:, j2, :],
                        iota[:],
                        ysl,
                        None,
                        mybir.AluOpType.is_equal,
                    )
                oh_tiles.append(oh_t)

            for j in range(kp):
                emb_t = emb_tiles[j]
                for q, (lo, n) in enumerate(CHUNKS):
                    nc.tensor.matmul(
                        psums[q][:],
                        lhsT=oh_tiles[j][:],
                        rhs=emb_t[:, :, lo : lo + n],
                        start=(j == 0),
                        stop=(j == kp - 1),
                        perf_mode=mybir.MatmulPerfMode.DoubleRow,
                    )

            # epilogue: r3 = 0.3/count, out = sums*r3 + 0.7*centroid.
            # scale-multiplies split across ACT (chunks 1, 0) and DVE
            # (chunk 2) in parallel, bf16 outputs so the DVE adds run in
            # 16-bit mode (2x); each output chunk ships as soon as its add
            # lands, split across the sync and scalar queues.
            r3 = fin_pool.tile([P, 1], _F32, name="r3")
            nc.vector.reciprocal(r3[:], psums[0][:, 128:129])
            nc.vector.tensor_scalar(
                r3[:], r3[:], FACTOR, None, mybir.AluOpType.mult
            )
            out_sb = fin_pool.tile([P, D], _BF16, name="out_sb")
            t1 = fin_pool.tile([P, 448], _BF16, name="t1")
            t2 = fin_pool.tile([P, 448], _BF16, name="t2")
            t0c = fin_pool.tile([P, 128], _BF16, name="t0c")
            # chunk 1 (cols 0:448) scale on ACT
            nc.scalar.mul(t1[:], psums[1][:, 0:448], r3[:, 0:1])
            # chunk 2 (cols 448:896) scale on DVE
            nc.vector.tensor_scalar(
                t2[:], psums[2][:, 0:448], r3[:, 0:1], None, mybir.AluOpType.mult
            )
            # chunk 0 (cols 896:1024) scale on ACT
            nc.scalar.mul(t0c[:], psums[0][:, 0:128], r3[:, 0:1])
            nc.vector.tensor_tensor(
                out=out_sb[:, 0:448],
                in0=t1[:],
                in1=cent_sb[:, 0:448],
                op=mybir.AluOpType.add,
            )
            nc.sync.dma_start(out=out[:, 0:448], in_=out_sb[:, 0:448])
            nc.vector.tensor_tensor(
                out=out_sb[:, 448:896],
                in0=t2[:],
                in1=cent_sb[:, 448:896],
                op=mybir.AluOpType.add,
            )
            nc.scalar.dma_start(out=out[:, 448:896], in_=out_sb[:, 448:896])
            nc.vector.tensor_tensor(
                out=out_sb[:, 896:D],
                in0=t0c[:],
                in1=cent_sb[:, 896:D],
                op=mybir.AluOpType.add,
            )
            nc.gpsimd.dma_start(out=out[:, 896:D], in_=out_sb[:, 896:D])

    nc.compile()
    return nc


def get_nc(cap: int = 4096):
    if cap not in _CACHE:
        _CACHE[cap] = _build(cap)
    return _CACHE[cap]


def _refine(groups, sums, counts, target):
    """2-opt repair: swap classes between the max bin and any other bin
    whenever it strictly lowers max(pair); stop at max <= target."""
    for _ in range(6000):
        hi = int(np.argmax(sums))
        if sums[hi] <= target:
            return True
        best = None  # (new_pair_max, ci, cj, b, d)
        for b in range(N_CORES):
            if b == hi:
                continue
            for ci in groups[hi]:
                for cj in groups[b]:
                    d = int(counts[ci]) - int(counts[cj])
                    if d <= 0:
                        continue
                    m = max(sums[hi] - d, sums[b] + d)
                    if m < sums[hi] and (best is None or m < best[0]):
                        best = (m, ci, cj, b, d)
        if best is None:
            return False
        _m, ci, cj, b, d = best
        groups[hi].remove(ci)
        groups[b].remove(cj)
        groups[hi].append(cj)
        groups[b].append(ci)
        sums[hi] -= d
        sums[b] += d
    return bool(np.max(sums) <= target)


def _partition_classes(counts: np.ndarray):
    """Split classes into N_CORES groups, <=128 classes each, minimizing
    the max total sample count. LPT greedy + 2-opt repair, with a few
    deterministic randomized restarts to reach a perfect equipartition."""
    target = int(np.ceil(counts.sum() / N_CORES))
    order = np.argsort(-counts, kind="stable")
    best_groups, best_sums = None, None
    for seed in range(8):
        rng = np.random.default_rng(seed)
        groups = [[] for _ in range(N_CORES)]
        sums = np.zeros(N_CORES, dtype=np.int64)
        for c in order:
            cand = np.argsort(
                sums + (rng.integers(0, 2, N_CORES) if seed else 0),
                kind="stable",
            )
            for b in cand:
                if len(groups[b]) < P:
                    groups[b].append(int(c))
                    sums[b] += counts[c]
                    break
        ok = _refine(groups, sums, counts, target)
        if best_sums is None or sums.max() < best_sums.max():
            best_groups, best_sums = groups, sums
        if ok:
            break
    return best_groups, best_sums


def make_in_maps(embed: np.ndarray, y: np.ndarray, centroid: np.ndarray):
    fp8_np = mybir.dt.np(_FP8)
    bf16_np = mybir.dt.np(_BF16)
    embed8 = np.ascontiguousarray(embed, dtype=np.float32).astype(fp8_np)
    y = np.asarray(y).astype(np.int64)
    centroid = np.asarray(centroid, dtype=np.float32)
    counts = np.bincount(y, minlength=C)

    groups, sums = _partition_classes(counts)
    cap = max(4096, int(np.ceil(sums.max() / 256.0)) * 256)

    # class -> (core, slot) map
    core_of = np.full(C, -1, dtype=np.int64)
    slot_of = np.full(C, -1, dtype=np.int64)
    for i, g in enumerate(groups):
        for s, cls in enumerate(g):
            core_of[cls] = i
            slot_of[cls] = s

    kt = cap // P
    in_maps = []
    meta = []
    for i in range(N_CORES):
        rows = np.nonzero(core_of[y] == i)[0]
        n = rows.shape[0]
        emb_pad = np.zeros((cap, W), dtype=fp8_np)
        emb_pad[:n, :D] = embed8[rows]
        emb_pad[:, D] = 1.0  # counts column (pad rows are masked by onehot)
        # cols D+1..W-1 stay zero (row alignment pad)
        ys = np.full(cap, -1.0, dtype=np.float32)
        ys[:n] = slot_of[y[rows]].astype(np.float32)
        # ysb[p, k] = slot label of padded row k*128+p, in f32
        ysb = np.ascontiguousarray(ys.reshape(kt, P).T)  # [P, kt] f32
        # emb8[p, k, :] = emb_pad[k*128 + p, :]
        emb8 = emb_pad.reshape(kt, P, W).transpose(1, 0, 2)
        # first tile: k-tiles 0,1 + all kt f32 labels in the pad tail.
        # sub-block j2 carries labels for k-tiles j2*16 .. j2*16+15.
        emb0 = np.zeros((P, 2, W0), dtype=fp8_np)
        emb0[:, :, :W] = emb8[:, 0:2, :]
        emb0[:, 0, W:W0] = ysb[:, 0:16].view(np.uint8).view(fp8_np)
        emb0[:, 1, W:W0] = ysb[:, 16:32].view(np.uint8).view(fp8_np)
        cent_i = np.zeros((P, D), dtype=np.float32)
        g = groups[i]
        cent_i[: len(g)] = (1.0 - FACTOR) * centroid[g]
        in_maps.append(
            {
                "emb0": emb0,
                "embr": np.ascontiguousarray(emb8[:, 2:, :]),
                "cent": cent_i.astype(bf16_np),
            }
        )
        meta.append(g)
    return in_maps, meta, cap


def kernel(embed: np.ndarray, y: np.ndarray, centroid: np.ndarray) -> np.ndarray:
    in_maps, meta, cap = make_in_maps(embed, y, centroid)
    nc = get_nc(cap)
    res = run_bass_kernel_spmd(nc, in_maps, core_ids=list(range(N_CORES)))
    full = np.zeros((C, D), dtype=np.float32)
    for i in range(N_CORES):
        g = meta[i]
        full[g] = res.results[i]["out"][: len(g)].astype(np.float32)
    return full
